# revision 11
# baseline (speedup 1.0000x reference)
"""Trainium2 Bass kernel for nn_AverageAttention.

Computation (per batch element b, L=4096 tokens, D=1024):
    avg   = cumsum(x, axis=tokens) / (t+1)            # cumulative average
    h     = LayerNorm(avg) (gamma/beta folded into w1/b1 on host)
    inter = relu(h @ w1 + b1)
    avg_o = inter @ w2 + b2 + avg
    gates = concat(x, avg_o) @ wg + bg
    out   = sigmoid(gates[:D]) * x + sigmoid(gates[D:]) * avg_o

Sharding: data-parallel over batch B=8 -> one batch element per NeuronCore.

Design notes:
 - x tiles loaded token-major [128 tok, 1024]; per-tile cumsum via an
   upper-triangular ones matmul on the PE in float32r (full rate at N>=256);
   the inter-tile carry is added with a rank-1 (K=1) matmul and chained by
   copying PSUM row 127 to SBUF on the scalar engine.
 - LN stats ride on scalar-engine accum_out; LN applied as per-partition
   scale/bias.
 - The 3 big matmuls run in bf16 with activations kept feature-major
   ([feature, token]); layout changes use DMA xbar transposes on bf16.
 - Weights pre-folded (ln_g/ln_b into w1/b1) and pre-cast to bf16 on host.
"""

import numpy as np
import ml_dtypes

B, L, D = 8, 4096, 1024
P = 128
NT = 256  # tokens per supertile (matmul moving free dim)

_CACHE = {}


def _build(L_=L):
    from contextlib import ExitStack

    import concourse.mybir as mybir
    import concourse.tile as tile
    from concourse import bacc
    from concourse.bass import ds, ts

    f32 = mybir.dt.float32
    f32r = mybir.dt.float32r
    bf16 = mybir.dt.bfloat16
    FT = mybir.ActivationFunctionType
    OP = mybir.AluOpType

    n_tiles = L_ // P
    n_st = L_ // NT
    SUB = NT // P
    KD = D // P        # 8 feature chunks for D
    KG = 2 * D // P    # 16 for the gating matmul
    H = D // 2         # 512: fp32 psum bank width

    nc = bacc.Bacc("TRN2", target_bir_lowering=False, debug=False, num_devices=8)

    x_d = nc.dram_tensor("x", [L_, D], f32, kind="ExternalInput").ap()
    w1_d = nc.dram_tensor("w1g", [D, D], bf16, kind="ExternalInput").ap()
    b1_d = nc.dram_tensor("b1f", [D], f32, kind="ExternalInput").ap()
    w2_d = nc.dram_tensor("w2b", [D, D], bf16, kind="ExternalInput").ap()
    b2_d = nc.dram_tensor("b2f", [D], f32, kind="ExternalInput").ap()
    wg_d = nc.dram_tensor("wgb", [2 * D, 2 * D], bf16, kind="ExternalInput").ap()
    bg_d = nc.dram_tensor("bgf", [2 * D], f32, kind="ExternalInput").ap()
    tri_d = nc.dram_tensor("triu", [P, P], f32, kind="ExternalInput").ap()
    ones_d = nc.dram_tensor("onesr", [1, P], f32, kind="ExternalInput").ap()
    rec_d = nc.dram_tensor("recip", [P, n_tiles], f32, kind="ExternalInput").ap()
    out_d = nc.dram_tensor("out", [L_, D], f32, kind="ExternalOutput").ap()

    with tile.TileContext(nc) as tc, ExitStack() as ctx:
        wpool = ctx.enter_context(tc.tile_pool(name="weights", bufs=1))
        xpool = ctx.enter_context(tc.tile_pool(name="xin", bufs=3))
        mpool = ctx.enter_context(tc.tile_pool(name="mid", bufs=2))
        spool = ctx.enter_context(tc.tile_pool(name="stats", bufs=4))
        apool = ctx.enter_context(tc.tile_pool(name="acts", bufs=2))
        gpool = ctx.enter_context(tc.tile_pool(name="gates", bufs=1))
        cpool = ctx.enter_context(tc.tile_pool(name="comb", bufs=3))
        opool = ctx.enter_context(tc.tile_pool(name="outs", bufs=2))
        cumpool = ctx.enter_context(tc.tile_pool(name="cum", bufs=2, space="PSUM"))
        cspool = ctx.enter_context(tc.tile_pool(name="cs", bufs=1, space="PSUM"))
        mmpool = ctx.enter_context(tc.tile_pool(name="mm", bufs=2, space="PSUM"))

        # ---- persistent weights / constants ----
        w1_sb = wpool.tile([P, KD, D], bf16)
        nc.sync.dma_start(w1_sb[:], w1_d.rearrange("(k p) m -> p k m", p=P))
        w2_sb = wpool.tile([P, KD, D], bf16)
        nc.sync.dma_start(w2_sb[:], w2_d.rearrange("(k p) m -> p k m", p=P))
        wg_sb = wpool.tile([P, KG, 2 * D], bf16)
        nc.sync.dma_start(wg_sb[:], wg_d.rearrange("(k p) m -> p k m", p=P))
        b1_sb = wpool.tile([P, KD], f32)
        nc.sync.dma_start(b1_sb[:], b1_d.rearrange("(f p) -> p f", p=P))
        b2_sb = wpool.tile([P, KD], f32)
        nc.sync.dma_start(b2_sb[:], b2_d.rearrange("(f p) -> p f", p=P))
        bg_sb = wpool.tile([P, KG], f32)
        nc.sync.dma_start(bg_sb[:], bg_d.rearrange("(f p) -> p f", p=P))
        tri_sb = wpool.tile([P, P], f32)
        nc.sync.dma_start(tri_sb[:], tri_d)
        ones_sb = wpool.tile([1, P], f32)
        nc.sync.dma_start(ones_sb[:], ones_d)
        onescol_sb = wpool.tile([P, 1], f32)
        nc.vector.memset(onescol_sb[:], 1.0)
        rec_sb = wpool.tile([P, n_tiles], f32)
        nc.sync.dma_start(rec_sb[:], rec_d)
        carry_sb = wpool.tile([1, D], f32)
        nc.vector.memset(carry_sb[:], 0.0)
        eps_sb = wpool.tile([P, 1], f32)
        nc.vector.memset(eps_sb[:], 1e-6)

        trir = tri_sb[:]
        onesr = ones_sb[:]
        carryr = carry_sb[:]

        for st in range(n_st):
            xT = apool.tile([P, KD, NT], bf16, tag="xT")
            hT = apool.tile([P, KD, NT], bf16, tag="hT")
            avT = apool.tile([P, KD, NT], bf16, tag="avT")

            # ---- phase A: load, cumsum, LN, transposes (per 128-token tile)
            for j in range(SUB):
                gi = st * SUB + j
                x_t = xpool.tile([P, D], f32, tag="x")
                nc.sync.dma_start(x_t[:], x_d[ts(gi, P)])
                xr = x_t[:]

                cps = cumpool.tile([P, D], f32, tag="cum")
                csum = cspool.tile([1, D], f32, tag="csum")
                for half in range(2):
                    sl = ds(half * H, H)
                    nc.tensor.matmul(cps[:, sl], trir, xr[:, sl],
                                     start=True, stop=False)
                    nc.tensor.matmul(cps[:, sl], onesr, carryr[:, sl],
                                     start=False, stop=True)
                    # tile column-sums (for the inter-tile carry chain)
                    nc.tensor.matmul(csum[:, sl], onescol_sb[:], xr[:, sl],
                                     start=True, stop=True)
                # carry += colsum(x_tile); next tile's carry matmul reads it
                nc.vector.tensor_add(carry_sb[:], carry_sb[:], csum[:])

                # avg (bf16) + row sums for LN stats
                ssum = spool.tile([P, 1], f32, tag="ssum")
                avg = mpool.tile([P, D], bf16, tag="avg")
                nc.scalar.activation(avg[:], cps[:], FT.Copy,
                                     scale=rec_sb[:, gi:gi + 1],
                                     accum_out=ssum[:])
                sq = mpool.tile([P, D], bf16, tag="sq")
                ssq = spool.tile([P, 1], f32, tag="ssq")
                nc.scalar.activation(sq[:], avg[:], FT.Square,
                                     accum_out=ssq[:])
                mu = spool.tile([P, 1], f32, tag="mu")
                nc.vector.tensor_scalar_mul(mu[:], ssum[:], 1.0 / D)
                musq = spool.tile([P, 1], f32, tag="musq")
                nc.vector.tensor_mul(musq[:], mu[:], mu[:])
                var = spool.tile([P, 1], f32, tag="var")
                nc.vector.scalar_tensor_tensor(var[:], ssq[:], 1.0 / D, musq[:],
                                               OP.mult, OP.subtract)
                std = spool.tile([P, 1], f32, tag="std")
                nc.scalar.activation(std[:], var[:], FT.Sqrt, bias=eps_sb[:])
                rstd = spool.tile([P, 1], f32, tag="rstd")
                nc.vector.reciprocal(rstd[:], std[:])
                nmr = spool.tile([P, 1], f32, tag="nmr")
                nc.vector.scalar_tensor_tensor(nmr[:], mu[:], -1.0, rstd[:],
                                               OP.mult, OP.mult)
                h_tm = mpool.tile([P, D], bf16, tag="h_tm")
                nc.scalar.activation(h_tm[:], avg[:], FT.Identity,
                                     scale=rstd[:], bias=nmr[:])
                x_bf = mpool.tile([P, D], bf16, tag="x_bf")
                nc.vector.tensor_copy(x_bf[:], x_t[:])

                tsl = ds(j * P, P)
                for c in range(KD):
                    csl = ds(c * P, P)
                    nc.sync.dma_start(xT[:, c, tsl], x_bf[:, csl], transpose=True)
                    nc.sync.dma_start(hT[:, c, tsl], h_tm[:, csl], transpose=True)
                    nc.sync.dma_start(avT[:, c, tsl], avg[:, csl], transpose=True)

            # ---- phase B: the three matmuls (feature-major) ----
            inT = apool.tile([P, KD, NT], bf16, tag="inT")
            for f in range(KD):
                ps = mmpool.tile([P, NT], f32, tag="mm")
                for k in range(KD):
                    nc.tensor.matmul(ps[:], w1_sb[:, k, ds(f * P, P)],
                                     hT[:, k, :],
                                     start=(k == 0), stop=(k == KD - 1))
                nc.scalar.activation(inT[:, f, :], ps[:], FT.Relu,
                                     bias=b1_sb[:, f:f + 1])

            aoT = apool.tile([P, KD, NT], bf16, tag="aoT")
            for f in range(KD):
                ps = mmpool.tile([P, NT], f32, tag="mm")
                for k in range(KD):
                    nc.tensor.matmul(ps[:], w2_sb[:, k, ds(f * P, P)],
                                     inT[:, k, :],
                                     start=(k == 0), stop=(k == KD - 1))
                nc.vector.scalar_tensor_tensor(aoT[:, f, :], ps[:],
                                               b2_sb[:, f:f + 1], avT[:, f, :],
                                               OP.add, OP.add)

            sg = gpool.tile([P, KG, NT], bf16, tag="sg")
            for f in range(KG):
                ps = mmpool.tile([P, NT], f32, tag="mm")
                for k in range(KG):
                    rhs = xT[:, k, :] if k < KD else aoT[:, k - KD, :]
                    nc.tensor.matmul(ps[:], wg_sb[:, k, ds(f * P, P)], rhs,
                                     start=(k == 0), stop=(k == KG - 1))
                nc.scalar.activation(sg[:, f, :], ps[:], FT.Sigmoid,
                                     bias=bg_sb[:, f:f + 1])

            # ---- combine + transpose back + store ----
            ot_tm = [opool.tile([P, D], bf16, tag=f"ot{j}", name=f"ot{j}")
                     for j in range(SUB)]
            for c in range(KD):
                t1 = cpool.tile([P, NT], bf16, tag="t1")
                t2 = cpool.tile([P, NT], bf16, tag="t2")
                oc = cpool.tile([P, NT], bf16, tag="oc")
                nc.vector.tensor_mul(t1[:], sg[:, c, :], xT[:, c, :])
                nc.vector.tensor_mul(t2[:], sg[:, c + KD, :], aoT[:, c, :])
                nc.vector.tensor_add(oc[:], t1[:], t2[:])
                for j in range(SUB):
                    nc.sync.dma_start(ot_tm[j][:, ds(c * P, P)],
                                      oc[:, ds(j * P, P)], transpose=True)
            for j in range(SUB):
                gi = st * SUB + j
                of = opool.tile([P, D], f32, tag="of")
                nc.scalar.copy(of[:], ot_tm[j][:])
                nc.sync.dma_start(out_d[ts(gi, P)], of[:])

    nc.compile()
    return nc


def _make_runner(nc, n_cores=8):
    """Build a cached jitted shard_map executor for the compiled Bass module
    (mirrors concourse.bass2jax.run_bass_via_pjrt, but reusable)."""
    import jax
    import concourse.mybir as mybir
    from concourse import bass2jax
    from jax.experimental.shard_map import shard_map
    from jax.sharding import Mesh, PartitionSpec

    bass2jax.install_neuronx_cc_hook()

    partition_name = (nc.partition_id_tensor.name
                      if nc.partition_id_tensor else None)
    in_names, out_names, out_avals, zero_outs = [], [], [], []
    for alloc in nc.m.functions[0].allocations:
        if not isinstance(alloc, mybir.MemoryLocationSet):
            continue
        name = alloc.memorylocations[0].name
        if alloc.kind == "ExternalInput":
            if name != partition_name:
                in_names.append(name)
        elif alloc.kind == "ExternalOutput":
            out_names.append(name)
            shape = tuple(alloc.tensor_shape)
            dtype = mybir.dt.np(alloc.dtype)
            out_avals.append(jax.core.ShapedArray(shape, dtype))
            zero_outs.append(np.zeros(shape, dtype))
    n_params = len(in_names)
    n_outs = len(out_avals)
    all_names = in_names + out_names
    if partition_name is not None:
        all_names = all_names + [partition_name]

    def _body(*args):
        operands = list(args)
        if partition_name is not None:
            operands.append(bass2jax.partition_id_tensor())
        outs = bass2jax._bass_exec_p.bind(
            *operands,
            out_avals=tuple(out_avals),
            in_names=tuple(all_names),
            out_names=tuple(out_names),
            lowering_input_output_aliases=(),
            sim_require_finite=True,
            sim_require_nnan=True,
            nc=nc,
        )
        return tuple(outs)

    devices = jax.devices()[:n_cores]
    mesh = Mesh(np.asarray(devices), ("core",))
    in_specs = (PartitionSpec("core"),) * (n_params + n_outs)
    out_specs = (PartitionSpec("core"),) * n_outs
    donate = tuple(range(n_params, n_params + n_outs))
    sharded = jax.jit(
        shard_map(_body, mesh=mesh, in_specs=in_specs, out_specs=out_specs,
                  check_rep=False),
        donate_argnums=donate, keep_unused=True,
    )

    def _concat(in_maps):
        concat_in = [
            np.concatenate([np.asarray(m[name]) for m in in_maps], axis=0)
            for name in in_names
        ]
        concat_zeros = [
            np.zeros((n_cores * z.shape[0], *z.shape[1:]), z.dtype)
            for z in zero_outs
        ]
        return concat_in, concat_zeros

    def run(in_maps):
        concat_in, concat_zeros = _concat(in_maps)
        out_arrs = sharded(*concat_in, *concat_zeros)
        return [
            {name: np.asarray(out_arrs[i]).reshape(n_cores, *out_avals[i].shape)[c]
             for i, name in enumerate(out_names)}
            for c in range(n_cores)
        ]

    def make_timed(in_maps):
        """Non-donating variant with device-resident inputs, for timing."""
        from jax.sharding import NamedSharding
        sharded_nd = jax.jit(
            shard_map(_body, mesh=mesh, in_specs=in_specs,
                      out_specs=out_specs, check_rep=False),
            keep_unused=True,
        )
        concat_in, concat_zeros = _concat(in_maps)
        sh = NamedSharding(mesh, PartitionSpec("core"))
        dev_args = [jax.device_put(a, sh) for a in concat_in + concat_zeros]
        jax.block_until_ready(dev_args)

        def timed_once():
            outs = sharded_nd(*dev_args)
            jax.block_until_ready(outs)
            return outs

        return timed_once

    run.make_timed = make_timed
    return run


def _prep_shared(w1, b1, w2, b2, ln_g, ln_b, wg, bg, L_=L):
    bf16 = ml_dtypes.bfloat16
    w1g = (np.asarray(w1, np.float32) * np.asarray(ln_g, np.float32)[:, None])
    b1f = (np.asarray(ln_b, np.float64) @ np.asarray(w1, np.float64)
           + np.asarray(b1, np.float64)).astype(np.float32)
    shared = {
        "w1g": np.ascontiguousarray(w1g.astype(bf16)),
        "b1f": b1f,
        "w2b": np.ascontiguousarray(np.asarray(w2, np.float32).astype(bf16)),
        "b2f": np.asarray(b2, np.float32),
        "wgb": np.ascontiguousarray(np.asarray(wg, np.float32).astype(bf16)),
        "bgf": np.asarray(bg, np.float32),
        "triu": np.triu(np.ones((P, P), np.float32)),
        "onesr": np.ones((1, P), np.float32),
        "recip": np.ascontiguousarray(
            (1.0 / (1.0 + np.arange(L_, dtype=np.float64)))
            .astype(np.float32).reshape(L_ // P, P).T),
    }
    return shared


def _get_runner(L_=L):
    key = ("runner", L_)
    if key not in _CACHE:
        nc = _build(L_)
        _CACHE[key] = _make_runner(nc)
    return _CACHE[key]


def kernel(inputs, w1, b1, w2, b2, ln_g, ln_b, wg, bg):
    inputs = np.asarray(inputs, dtype=np.float32)
    Bi, Li, Di = inputs.shape
    assert (Bi, Li, Di) == (B, L, D), (Bi, Li, Di)
    run = _get_runner(L)
    shared = _prep_shared(w1, b1, w2, b2, ln_g, ln_b, wg, bg, L)
    in_maps = [dict(shared, x=np.ascontiguousarray(inputs[b])) for b in range(B)]
    results = run(in_maps)
    return np.stack([results[b]["out"] for b in range(B)], axis=0)
